# revision 2
# baseline (speedup 1.0000x reference)
"""Trainium2 Bass kernel for nn_PlainTensorProduct (L_MAX=3, B=2048, F=128).

Pure data parallel over 8 NeuronCores (256 batch rows each).  Per core:

  out[b, r3*F+f] = sum_t coef_t * x[b, c1_t, f] * y[b, c2_t, f]

with 40 flattened input components per side and 247 output rows; the
sym/detrace/normalization of every (l1,l2)->l3 path is folded into the
constant coefficient matrix W2 (built here in numpy).

On-chip pipeline (components on SBUF partitions, columns = (batch, feat)):
  GPSIMD: z0 = broadcast(x0) * Y                      (40 x0*y products)
  PE:     Xrep/Yrep = SEL.T @ X/Y -> PSUM             (pair replication)
  ACT:    evacuate Xrep PSUM -> SBUF (shared by 2 product chunks)
  DVE:    z_k = Xrep * Yrep                           (4 chunks of <=128)
  PE:     outA/outB (PSUM) += W2 blocks @ z           (float32r matmuls)
  ACT:    evacuate out PSUM -> SBUF;  DMA -> HBM

Matmuls run in float32r (TF32-like) — full PE rate; end-to-end rel err
~2e-4 versus the fp32 reference.
"""

import itertools
from contextlib import ExitStack

import numpy as np

L_MAX = 3
BASES = [0, 1, 4, 13]
NCOMP = 40
F = 128
NROWS = 247
CHUNK = 128
B_FULL = 2048
N_CORES = 8
B_CORE = B_FULL // N_CORES

XPART = [64, 65, 68, 77]       # x comp partition bases (l=0..3)
YPART = [0, 1, 4, 13]          # y comp partition bases


# ---------------------------------------------------------------------------
# constant (coefficient / selection) matrix construction
# ---------------------------------------------------------------------------

def _dfac(n):
    x = 1
    for i in range(1, n + 1, 2):
        x *= i
    return x


def _fac(n):
    x = 1
    for i in range(1, n + 1):
        x *= i
    return x


def _norm(l1, l2, l3):
    J = l1 + l2 + l3
    J1, J2, J3 = J - 2 * l1 - 1, J - 2 * l2 - 1, J - 2 * l3 - 1
    num = (_fac(l1) * _fac(l2) * _dfac(2 * l3 - 1)
           * _fac((J1 + 1) // 2) * _fac((J2 + 1) // 2))
    den = _fac(l3) * _dfac(J1) * _dfac(J2) * _dfac(J3) * _fac(J // 2)
    return num / den


def _paths():
    out = {}
    for l3 in range(L_MAX + 1):
        p = []
        for l1 in range(L_MAX + 1):
            for l2 in range(L_MAX + 1):
                s = l1 + l2 - l3
                if s >= 0 and s % 2 == 0 and s // 2 <= min(l1, l2):
                    p.append((l1, l2))
        out[l3] = p
    return out


def _sym_detrace_np(t, l3):
    if l3 < 2:
        return t
    if l3 == 2:
        t = 0.5 * (t + np.swapaxes(t, 1, 2))
        tr = np.einsum('baaf->bf', t)
        return t - np.eye(3)[None, :, :, None] * (tr[:, None, None, :] / 3.0)
    perms = list(itertools.permutations((1, 2, 3)))
    t = sum(np.transpose(t, (0,) + p + (4,)) for p in perms) / len(perms)
    v = np.einsum('baccf->baf', t)
    d = np.eye(3)
    corr = (d[None, :, :, None, None] * v[:, None, None, :, :]
            + d[None, :, None, :, None] * v[:, None, :, None, :]
            + d[None, None, :, :, None] * v[:, :, None, None, :])
    return t - corr / 5.0


def _sym_detrace_mat(l3):
    n = 3 ** l3
    S = np.zeros((n, n))
    for col in range(n):
        t = np.zeros((1,) + (3,) * l3 + (1,))
        t.reshape(1, n, 1)[0, col, 0] = 1.0
        S[:, col] = _sym_detrace_np(t, l3).reshape(n)
    return S


def _path_list():
    out = []
    row = 0
    for l3 in range(L_MAX + 1):
        for (l1, l2) in _paths()[l3]:
            out.append((l1, l2, l3, (l1 + l2 - l3) // 2, row))
            row += 3 ** l3
    assert row == NROWS
    return out


def _term_dict():
    terms = {}
    for (l1, l2, l3, k, row0) in _path_list():
        S = _sym_detrace_mat(l3) * _norm(l1, l2, l3)
        nA, nM, nC = 3 ** (l1 - k), 3 ** k, 3 ** (l2 - k)
        for a in range(nA):
            for m in range(nM):
                for c in range(nC):
                    c1 = BASES[l1] + a * nM + m
                    c2 = BASES[l2] + m * nC + c
                    col = S[:, a * nC + c]
                    lst = terms.setdefault((c1, c2), [])
                    for ci in np.nonzero(col)[0]:
                        lst.append((row0 + ci, col[ci]))
    merged = {}
    for key, lst in terms.items():
        acc = {}
        for r, c in lst:
            acc[r] = acc.get(r, 0.0) + c
        merged[key] = {r: c for r, c in acc.items() if c != 0.0}
    return merged


def build_constants():
    terms = _term_dict()
    gp = {c2: v for (c1, c2), v in terms.items() if c1 == 0}
    dv = {(c1, c2): v for (c1, c2), v in terms.items() if c1 != 0}

    def cls(v):
        a = any(r < 128 for r in v)
        b = any(r >= 128 for r in v)
        return 0 if (a and not b) else (1 if (a and b) else 2)

    by_c1 = {}
    for (c1, c2), v in sorted(dv.items()):
        by_c1.setdefault(c1, []).append((c2, v))
    groups = []
    for c1, lst in sorted(by_c1.items()):
        a = [e for e in lst if cls(e[1]) == 0]
        m = [e for e in lst if cls(e[1]) == 1]
        b = [e for e in lst if cls(e[1]) == 2]
        while len(a) >= 2:
            groups.append((3, c1, a.pop(0), a.pop(0)))
        while len(b) >= 2:
            groups.append((1, c1, b.pop(0), b.pop(0)))
        if a and b:
            groups.append((0, c1, a.pop(0), b.pop(0)))
        rest = a + m + b
        while len(rest) >= 2:
            groups.append((2, c1, rest.pop(0), rest.pop(0)))
        if rest:
            groups.append((4, c1, rest.pop(0), None))
    groups.sort(key=lambda g: (g[0], g[1]))
    npair = (len(groups) + CHUNK - 1) // CHUNK
    nch = 2 * npair

    SELX = np.zeros((NCOMP, npair * CHUNK), np.float32)
    SELY = np.zeros((NCOMP, nch * CHUNK), np.float32)
    W2D = np.zeros((nch * CHUNK, NROWS), np.float32)
    for g, (_, c1, pa, pb) in enumerate(groups):
        j, p = g // CHUNK, g % CHUNK
        SELX[c1, j * CHUNK + p] = 1.0
        for half, prod in ((0, pa), (1, pb)):
            if prod is None:
                continue
            c2, v = prod
            kc = 2 * j + half
            SELY[c2, kc * CHUNK + p] = 1.0
            for r, c in v.items():
                W2D[kc * CHUNK + p, r] = np.float32(c)

    W2G = np.zeros((64, NROWS), np.float32)
    for c2, v in gp.items():
        for r, c in v.items():
            W2G[c2, r] = np.float32(c)
    assert np.all(W2G[:, 128:] == 0.0)

    blocks = []
    for kc in range(nch):
        for mc, (r0, r1) in enumerate(((0, 128), (128, NROWS))):
            if np.any(W2D[kc * CHUNK:(kc + 1) * CHUNK, r0:r1] != 0):
                blocks.append(("dve", kc, mc))
    if np.any(W2G[:, :128] != 0):
        blocks.append(("gp", 0, 0))
    return dict(SELX=SELX, SELY=SELY, W2D=W2D, W2G=W2G, blocks=blocks,
                nch=nch, npair=npair)


# ---------------------------------------------------------------------------
# bass program
# ---------------------------------------------------------------------------

def _last_for_mc(blocks, j, mc):
    return all(b[2] != mc for b in blocks[j + 1:])


def build_program(consts, B_core=B_CORE, FD=512, BBLOCK=32):
    import concourse.bacc as bacc
    import concourse.tile as tile
    import concourse.mybir as mybir

    nch = consts["nch"]
    npair = consts["npair"]
    blocks = consts["blocks"]
    nblk = len(blocks)

    nc = bacc.Bacc(None, target_bir_lowering=False, debug=False)
    fp32 = mybir.dt.float32
    ddt = mybir.dt.float32r

    x_dram = [nc.dram_tensor(f"in_x{l}", [B_core, 3 ** l * F], ddt,
                             kind="ExternalInput") for l in range(4)]
    y_dram = [nc.dram_tensor(f"in_y{l}", [B_core, 3 ** l * F], ddt,
                             kind="ExternalInput") for l in range(4)]
    selxy_dram = nc.dram_tensor("in_selxy", [128, (npair + nch) * CHUNK],
                                ddt, kind="ExternalInput")
    w2_dram = nc.dram_tensor("in_w2", [128, nblk * CHUNK], ddt,
                             kind="ExternalInput")
    out_dram = nc.dram_tensor("out_tp", [B_core, NROWS * F], fp32,
                              kind="ExternalOutput")

    with tile.TileContext(nc) as tc, ExitStack() as ctx:
        consts_pool = ctx.enter_context(tc.tile_pool(name="consts", bufs=1))
        in_pool = ctx.enter_context(tc.tile_pool(name="inp", bufs=2))
        gp_pool = ctx.enter_context(tc.tile_pool(name="gp", bufs=2))
        z_pool = ctx.enter_context(tc.tile_pool(name="z", bufs=2))
        st_pool = ctx.enter_context(tc.tile_pool(name="st", bufs=3))
        rep_pool = ctx.enter_context(
            tc.tile_pool(name="rep", bufs=2, space="PSUM"))
        xrep_pool = ctx.enter_context(
            tc.tile_pool(name="xrep", bufs=2, space="PSUM"))
        acc_pool = ctx.enter_context(
            tc.tile_pool(name="acc", bufs=2, space="PSUM"))

        selxy = consts_pool.tile([128, (npair + nch) * CHUNK], ddt)
        nc.sync.dma_start(selxy[:], selxy_dram[:])
        w2 = consts_pool.tile([128, nblk * CHUNK], ddt)
        nc.sync.dma_start(w2[:], w2_dram[:])

        for bb in range(B_core // BBLOCK):
            bsl = slice(bb * BBLOCK, (bb + 1) * BBLOCK)
            xy = in_pool.tile([128, BBLOCK * F], ddt, tag="xy")
            x0t = in_pool.tile([1, BBLOCK * F], ddt, tag="x0t")
            nc.sync.dma_start(
                out=x0t[0:1, :].rearrange("c (b f) -> c b f", f=F),
                in_=x_dram[0][bsl, :].rearrange("b (c f) -> c b f", f=F))
            for l in range(4):
                w = 3 ** l
                nc.sync.dma_start(
                    out=xy[XPART[l]:XPART[l] + w, :].rearrange(
                        "c (b f) -> c b f", f=F),
                    in_=x_dram[l][bsl, :].rearrange("b (c f) -> c b f", f=F))
                nc.sync.dma_start(
                    out=xy[YPART[l]:YPART[l] + w, :].rearrange(
                        "c (b f) -> c b f", f=F),
                    in_=y_dram[l][bsl, :].rearrange("b (c f) -> c b f", f=F))

            x0b = in_pool.tile([40, BBLOCK * F], ddt, tag="x0b")
            nc.gpsimd.partition_broadcast(x0b[0:40, :], x0t[0:1, :])

            for nb in range(BBLOCK * F // FD):
                csl = slice(nb * FD, (nb + 1) * FD)
                z0 = gp_pool.tile([128, FD], ddt, tag="z0")
                nc.gpsimd.tensor_tensor(
                    z0[0:40, :], x0b[0:40, csl], xy[0:40, csl],
                    mybir.AluOpType.mult)

                zd = []
                for j in range(npair):
                    jsl = slice(j * CHUNK, (j + 1) * CHUNK)
                    xs = z_pool.tile([128, FD], fp32, tag="xs")
                    xr = xrep_pool.tile([128, FD], fp32, tag="xr")
                    nc.tensor.matmul(
                        xr[:], selxy[64:104, jsl], xy[64:104, csl],
                        start=True, stop=True, tile_position=(64, 0))
                    nc.scalar.copy(xs[:], xr[:])
                    for half in range(2):
                        kc = 2 * j + half
                        ksl = slice((npair + kc) * CHUNK,
                                    (npair + kc + 1) * CHUNK)
                        yr = rep_pool.tile([128, FD], fp32, tag="yr")
                        nc.tensor.matmul(
                            yr[:], selxy[0:40, ksl], xy[0:40, csl],
                            start=True, stop=True, tile_position=(0, 0))
                        z = z_pool.tile([128, FD], ddt, tag=f"z{kc}")
                        nc.vector.tensor_tensor(
                            z[:], xs[:], yr[:], mybir.AluOpType.mult)
                        zd.append(z)

                outA = acc_pool.tile([128, FD], fp32, tag="outA")
                outB = acc_pool.tile([NROWS - 128, FD], fp32, tag="outB")
                startedA = False
                startedB = False
                for j, (src, kc, mc) in enumerate(blocks):
                    m = 128 if mc == 0 else NROWS - 128
                    jsl = slice(j * CHUNK, j * CHUNK + m)
                    if src == "dve":
                        lhsT, rhs, tp = w2[0:128, jsl], zd[kc][:], (0, 0)
                    else:
                        lhsT, rhs, tp = w2[0:40, jsl], z0[0:40, :], (0, 0)
                    if mc == 0:
                        o, started, startedA = outA, startedA, True
                    else:
                        o, started, startedB = outB, startedB, True
                    nc.tensor.matmul(
                        o[:], lhsT, rhs, start=not started,
                        stop=_last_for_mc(blocks, j, mc), tile_position=tp)

                b0 = bb * BBLOCK + nb * FD // F
                nbr = FD // F
                oA = st_pool.tile([128, FD], fp32, tag="oA")
                nc.scalar.copy(oA[:], outA[:])
                nc.sync.dma_start(
                    out=out_dram[b0:b0 + nbr, 0:128 * F].rearrange(
                        "b (r f) -> r b f", f=F),
                    in_=oA[:].rearrange("r (b f) -> r b f", f=F))
                oB = st_pool.tile([NROWS - 128, FD], fp32, tag="oB")
                nc.scalar.copy(oB[:], outB[:])
                nc.sync.dma_start(
                    out=out_dram[b0:b0 + nbr, 128 * F:].rearrange(
                        "b (r f) -> r b f", f=F),
                    in_=oB[:].rearrange("r (b f) -> r b f", f=F))

    nc.compile()
    names = dict(
        x=[t.name for t in x_dram], y=[t.name for t in y_dram],
        selxy=selxy_dram.name, w2=w2_dram.name, out=out_dram.name)
    return nc, names


def _make_const_arrays(consts):
    nch, npair, blocks = consts["nch"], consts["npair"], consts["blocks"]
    selxy = np.zeros((128, (npair + nch) * CHUNK), np.float32)
    selxy[64:104, 0:npair * CHUNK] = consts["SELX"]
    selxy[0:40, npair * CHUNK:] = consts["SELY"]
    w2 = np.zeros((128, len(blocks) * CHUNK), np.float32)
    for j, (src, kc, mc) in enumerate(blocks):
        r0, r1 = (0, 128) if mc == 0 else (128, NROWS)
        m = r1 - r0
        if src == "dve":
            w2[0:128, j * CHUNK:j * CHUNK + m] = \
                consts["W2D"][kc * CHUNK:(kc + 1) * CHUNK, r0:r1]
        else:
            w2[0:40, j * CHUNK:j * CHUNK + m] = consts["W2G"][0:40, r0:r1]
    return selxy, w2


_CACHE = {}


def _get_program():
    if "prog" not in _CACHE:
        consts = build_constants()
        nc, names = build_program(consts)
        selxy, w2 = _make_const_arrays(consts)
        _CACHE["prog"] = (consts, nc, names, selxy, w2)
    return _CACHE["prog"]


def kernel(x0, x1, x2, x3, y0, y1, y2, y3):
    from concourse import bass_utils

    consts, nc, names, selxy, w2 = _get_program()
    xs = [np.ascontiguousarray(np.asarray(a, np.float32).reshape(B_FULL, -1))
          for a in (x0, x1, x2, x3)]
    ys = [np.ascontiguousarray(np.asarray(a, np.float32).reshape(B_FULL, -1))
          for a in (y0, y1, y2, y3)]
    in_maps = []
    for c in range(N_CORES):
        sl = slice(c * B_CORE, (c + 1) * B_CORE)
        m = {names["selxy"]: selxy, names["w2"]: w2}
        for l in range(4):
            m[names["x"][l]] = xs[l][sl]
            m[names["y"][l]] = ys[l][sl]
        in_maps.append(m)
    res = bass_utils.run_bass_kernel_spmd(
        nc, in_maps, core_ids=list(range(N_CORES)))
    out = np.empty((B_FULL, NROWS * F), np.float32)
    for c in range(N_CORES):
        out[c * B_CORE:(c + 1) * B_CORE] = res.results[c][names["out"]]
    return out
